# revision 59
# baseline (speedup 1.0000x reference)
# Trainium2 Bass kernel for nn_Attention_63900523430102.
#
# Reference computes, for q,k,v of shape (S=2048, B=4, D=1024):
#   xq = q @ wq.T, xk = k @ wk.T, xv = v @ wv.T  (per-head split, hd=64, H=16)
#   xq, xk = rope(xq), rope(xk)
#   scores = softmax(xq . xk / sqrt(hd)) ; out = (scores @ xv) @ wo.T
#
# Sharding: 8 cores = 4 batches x 2 head-groups (8 heads each).  Host
# pretransposes operands so projections emit xqT/xkT [hd, S] directly.
#
# This version (vs the 525us baseline):
#  - scores matmuls are K=64 row-tiled: the two heads of a pair live in
#    partitions 0-63 / 64-127, and their score matmuls are emitted
#    back-to-back so the PE runs them CONCURRENTLY on the two halves of
#    the array (tile_position (0,0) / (64,0)) -> 2x score throughput.
#  - softmax exp is split across ACT (exact Exp) and DVE (Schraudolph
#    int16-bitcast exp approx, one tensor_scalar op) -- ACT alone is a
#    ~285us wall.
#  - phase order k-proj -> v-proj -> q-proj so attention starts as soon
#    as xq(sb0) exists; WO is folded per-sb reusing the score PSUM pool.
#  - batched DMAs (one trigger per [128, 4096] block) to cut trigger cost.
import sys
import os

sys.path.insert(0, "/opt/trn_rl_repo")

import numpy as np
import ml_dtypes

import concourse.bass as bass
import concourse.bacc as bacc
import concourse.mybir as mybir
import concourse.tile as tile
from concourse.bass_utils import run_bass_kernel_spmd


def _shim_axon_hooks():
    """Provide antenv.axon_hooks (NTFF profile hook) if the image lacks it."""
    try:
        from antenv.axon_hooks import get_axon_ntff_profile_hook  # noqa: F401
        return
    except ImportError:
        pass
    import types
    import ctypes
    import contextlib

    so_path = "/opt/axon/libaxon_pjrt.so"
    hook = None
    if os.path.exists(so_path):
        lib = ctypes.CDLL(so_path)
        if hasattr(lib, "axon_start_nrt_profile"):
            lib.axon_start_nrt_profile.argtypes = [
                ctypes.POINTER(ctypes.c_int64), ctypes.c_size_t]
            lib.axon_start_nrt_profile.restype = ctypes.c_int64
            lib.axon_stop_nrt_profile.argtypes = [ctypes.c_char_p]
            lib.axon_stop_nrt_profile.restype = ctypes.c_int64

            @contextlib.contextmanager
            def hook(output_dir, device_ids):
                import jax
                jax.devices()
                if device_ids:
                    ids = (ctypes.c_int64 * len(device_ids))(*device_ids)
                    rc = lib.axon_start_nrt_profile(ids, len(device_ids))
                else:
                    rc = lib.axon_start_nrt_profile(None, 0)
                if rc != 0:
                    raise RuntimeError(f"axon_start_nrt_profile rc={rc}")
                try:
                    yield
                finally:
                    n = lib.axon_stop_nrt_profile(str(output_dir).encode())
                    print(f"ntff profile: {n} file(s) -> {output_dir}",
                          file=sys.stderr)

    mod = types.ModuleType("antenv.axon_hooks")
    mod.get_axon_ntff_profile_hook = lambda: hook
    mod.set_axon_ntff_profile_hook = lambda h: None
    sys.modules["antenv.axon_hooks"] = mod


_shim_axon_hooks()

S = 2048          # seq len (both query s and key l)
D = 1024          # d_model
B = 4             # batch
HLOC = 8          # heads per core
HD = 64           # head dim
E = HLOC * HD     # 512, local e-width per core
NCORES = 8
NPAIR = 4         # head pairs per core (2 heads stacked -> 128 partitions)
NSB = 4           # s blocks of 512
NLT = 16          # l tiles of 128
ND = 8            # d_model tiles of 128

BF16 = mybir.dt.bfloat16
F32 = mybir.dt.float32
I16 = mybir.dt.int16
NPBF16 = ml_dtypes.bfloat16

# Schraudolph exp on DVE: pr = exp(0.125*sc) ~= bf16_bits(round(A*sc + B)).
# int16 value I interpreted as bf16 is 2^((I-16256)/128)*(1+frac), linear
# Schraudolph; B tuned for minimax relative error (~3.3% max, 2.0% rms).
EXP_A = 23.083120654223414
EXP_B = 16250.375
# l-tiles whose (both-heads) exp runs on DVE via Schraudolph; the rest on
# ACT.  7/16 keeps the approx share at 43.75% (rel err ~0.0166 measured).
DVE_LT = (1, 3, 5, 7, 9, 11, 13)

DEBUG_DUMPS = False

_PROG = None
LAST_RESULT = None


def _emit(nc, tc, aps):
    qT, kT, vT, wqT, wkT, wvT, woT, ctab, stab, out = aps[:10]
    Exp = mybir.ActivationFunctionType.Exp
    Copy = mybir.ActivationFunctionType.Copy
    MUL = mybir.AluOpType.mult
    ADD = mybir.AluOpType.add
    swap_mask = [i ^ 1 for i in range(32)]

    from contextlib import ExitStack

    with ExitStack() as stk:
        consts = stk.enter_context(tc.tile_pool(name="consts", bufs=1))
        persist = stk.enter_context(tc.tile_pool(name="persist", bufs=1))

        # ---- persistent constants ----
        # wk on sync, k tiles on gpsimd so the first projections' two
        # operands stream in parallel.
        wk_all = consts.tile([128, ND * E], BF16, tag="wk", name="wk")
        wv_all = consts.tile([128, ND * E], BF16, tag="wv", name="wv")
        wq_all = consts.tile([128, ND * E], BF16, tag="wq", name="wq")
        wo_all = consts.tile([128, NPAIR * D], BF16, tag="wo", name="wo")
        ct_t = consts.tile([128, S], BF16, tag="ct", name="ct")
        st2_t = consts.tile([128, S], BF16, tag="st2", name="st2")

        def load_w(dst, src, eng, nsplit=2):
            # dst [128, 8*512] <- src [1024, 512] (8 d-tiles), nsplit triggers
            d3 = dst.rearrange("p (d e) -> p d e", e=E)
            s3 = src.rearrange("(d p) e -> p d e", p=128)
            step = ND // nsplit
            for h in range(nsplit):
                eng.dma_start(out=d3[:, h * step:(h + 1) * step],
                              in_=s3[:, h * step:(h + 1) * step])

        load_w(wk_all, wkT, nc.sync)
        nc.sync.dma_start(out=ct_t, in_=ctab[:, :])
        nc.sync.dma_start(out=st2_t, in_=stab[:, :])
        load_w(wv_all, wvT, nc.sync)
        load_w(wq_all, wqT, nc.sync)

        # ---- persistent activations ----
        xq_sb = [persist.tile([128, S], BF16, tag=f"xq{p}", name=f"xq{p}")
                 for p in range(NPAIR)]
        xk_sb = [persist.tile([128, S], BF16, tag=f"xk{p}", name=f"xk{p}")
                 for p in range(NPAIR)]
        # xv with a ones column per head: head h occupies cols 65h..65h+64
        xv_sb = [persist.tile([128, HLOC * (HD + 1)], BF16,
                              tag=f"xv{lt}", name=f"xv{lt}")
                 for lt in range(NLT)]
        attT = [persist.tile([128, S], BF16, tag=f"att{p}", name=f"att{p}")
                for p in range(NPAIR)]

        # warm up the gpsimd partition_broadcast ucode: its very first
        # invocation on a cold core can signal completion before the write
        # lands (observed as intermittent first-call corruption).  Nothing
        # consumes wrm_b, so the real broadcasts all run with the ucode
        # resident.
        wrm_a = persist.tile([1, 512], F32, tag="wrm_a", name="wrm_a")
        wrm_b = persist.tile([64, 512], F32, tag="wrm_b", name="wrm_b")
        nc.vector.memset(wrm_a, 1.0)
        nc.gpsimd.partition_broadcast(wrm_b, wrm_a)

        # =============== Phase A: projections + rope ===============
        with ExitStack() as phase_a:
            kq_pool = phase_a.enter_context(tc.tile_pool(name="kq", bufs=4))
            v_pool = phase_a.enter_context(tc.tile_pool(name="vp", bufs=4))
            rp = phase_a.enter_context(tc.tile_pool(name="rope", bufs=2))
            psA = phase_a.enter_context(
                tc.tile_pool(name="psA", bufs=4, space="PSUM"))
            psV = phase_a.enter_context(
                tc.tile_pool(name="psV", bufs=3, space="PSUM"))

            def rope(ps, dst, sb):
                # dst[p] = ps[p]*ct[p] + ps[p^1]*st[p]; st2 = row-swapped st
                cols = slice(sb * 512, (sb + 1) * 512)
                xb = rp.tile([128, 512], BF16, tag="xb", name="xb")
                nc.scalar.activation(xb, ps, Copy)
                t1 = rp.tile([128, 512], BF16, tag="t1", name="t1")
                nc.vector.tensor_mul(t1, xb, ct_t[:, cols])
                t2 = rp.tile([128, 512], BF16, tag="t2", name="t2")
                nc.vector.tensor_mul(t2, xb, st2_t[:, cols])
                sw = rp.tile([128, 512], BF16, tag="sw", name="sw")
                nc.vector.stream_shuffle(sw, t2, swap_mask)
                nc.vector.tensor_add(dst[:, cols], t1, sw)

            def proj_qk(src_dram, w_all, dst_sb, dma_eng):
                for sb in range(NSB):
                    scol = slice(sb * 512, (sb + 1) * 512)
                    t = kq_pool.tile([128, ND * 512], BF16, tag="kq",
                                     name="kq")
                    d3 = t.rearrange("p (d e) -> p d e", e=512)
                    s3 = src_dram.rearrange("(d p) s -> p d s", p=128)
                    for h in range(2):
                        dma_eng.dma_start(
                            out=d3[:, h * 4:(h + 1) * 4],
                            in_=s3[:, h * 4:(h + 1) * 4, scol])
                    for p in range(NPAIR):
                        pc0 = p * 128
                        ps = psA.tile([128, 512], F32, tag="ps", name="ps")
                        for d in range(ND):
                            nc.tensor.matmul(
                                ps,
                                lhsT=w_all[:, d * E + pc0:d * E + pc0 + 128],
                                rhs=t[:, d * 512:(d + 1) * 512],
                                start=(d == 0), stop=(d == ND - 1))
                        rope(ps, dst_sb[p], sb)

            # k first (phase B sb0 needs all xk), then q, then v LAST:
            # the phase-A teardown barrier then waits only on the cheap v
            # drains instead of the 7us rope tail, and phase B's PV(g)
            # anyway consumes xv tiles in lt order.
            proj_qk(kT, wk_all, xk_sb, nc.gpsimd)
            proj_qk(qT, wq_all, xq_sb, nc.sync)

            for lt in range(NLT):
                lcol = slice(lt * 128, (lt + 1) * 128)
                vt = v_pool.tile([128, ND * 128], BF16, tag="v", name="v")
                nc.gpsimd.dma_start(
                    out=vt.rearrange("p (d l) -> p d l", l=128),
                    in_=vT.rearrange("(d p) s -> p d s", p=128)[:, :, lcol])
                xv_ps = psV.tile([128, 512], F32, tag="xv", name="xv")
                for d in range(ND):
                    nc.tensor.matmul(xv_ps,
                                     lhsT=vt[:, d * 128:(d + 1) * 128],
                                     rhs=wv_all[:, d * E:(d + 1) * E],
                                     start=(d == 0), stop=(d == ND - 1))
                dst = xv_sb[lt].rearrange("p (h c) -> p h c", c=HD + 1)
                src = xv_ps.rearrange("p (h c) -> p h c", c=HD)
                nc.vector.tensor_copy(dst[:, :, 0:HD], src)
                nc.vector.memset(dst[:, :, HD], 1.0)

        # wo load deferred: first consumed at the first WO round
        woc = wo_all.rearrange("p (e d) -> p e d", d=D)
        nc.sync.dma_start(out=woc, in_=woT.rearrange("(e p) d -> p e d", p=128))

        # ====== Phase B: attention (sb-major) with WO folded in ======
        # PSUM: scB 3x[128,1024] = 6 banks + pv0/pv1 = 2 banks.  WO reuses
        # the scB pool between sb rounds.
        with ExitStack() as phase_b:
            scB = phase_b.enter_context(
                tc.tile_pool(name="scB", bufs=3, space="PSUM"))
            pvP = phase_b.enter_context(
                tc.tile_pool(name="pvP", bufs=1, space="PSUM"))
            pra = phase_b.enter_context(tc.tile_pool(name="pra", bufs=4))
            prb = phase_b.enter_context(tc.tile_pool(name="prb", bufs=3))
            smal = phase_b.enter_context(tc.tile_pool(name="smal", bufs=2))
            otp = phase_b.enter_context(tc.tile_pool(name="otp", bufs=2))

            def emit_wo(sti):
                trow = slice(sti * 128, (sti + 1) * 128)
                ps = scB.tile([128, 1024], F32, tag="sc", name="wops")
                for nb in range(2):
                    nbc = slice(nb * 512, (nb + 1) * 512)
                    for et in range(NPAIR):
                        nc.tensor.matmul(
                            ps[:, nbc],
                            lhsT=attT[et][:, trow],
                            rhs=wo_all[:, et * D + nb * 512:
                                       et * D + nb * 512 + 512],
                            start=(et == 0), stop=(et == NPAIR - 1))
                ot = otp.tile([128, 1024], F32, tag="ot", name="ot")
                if sti % 2 == 0:
                    nc.scalar.activation(ot, ps, Copy)
                else:
                    nc.vector.tensor_copy(ot, ps)
                eng = nc.sync if sti % 2 == 0 else nc.gpsimd
                eng.dma_start(out=out[trow, :], in_=ot)

            for sb in range(NSB):
                scol = slice(sb * 512, (sb + 1) * 512)
                for p in range(NPAIR):
                    h0, h1 = 2 * p, 2 * p + 1
                    pv0 = pvP.tile([128, 512], F32, tag="pv0", name="pv0")
                    pv1 = pvP.tile([128, 512], F32, tag="pv1", name="pv1")

                    def emit_pv(pr, lt):
                        nc.tensor.matmul(
                            pv0[0:HD + 1, :],
                            lhsT=xv_sb[lt][:, 65 * h0:65 * h0 + 65],
                            rhs=pr[:, 0:512],
                            start=(lt == 0), stop=(lt == NLT - 1))
                        nc.tensor.matmul(
                            pv1[0:HD + 1, :],
                            lhsT=xv_sb[lt][:, 65 * h1:65 * h1 + 65],
                            rhs=pr[:, 512:1024],
                            start=(lt == 0), stop=(lt == NLT - 1))

                    # ONE [128,1024] score tile per l-tile holds BOTH heads
                    # (h0 cols 0-511, h1 cols 512-1023): one tile per group
                    # triples the 3-buffer rotation slack vs two-tiles-per-
                    # group, so exp jitter and the WO buffer theft no longer
                    # break the row-tile matmul concurrency.
                    prev = None
                    for lt in range(NLT):
                        lcol = slice(lt * 128, (lt + 1) * 128)
                        sct = scB.tile([128, 1024], F32, tag="sc",
                                       name="sct")
                        for hh in range(2):
                            hr = slice(64 * hh, 64 * hh + 64)
                            hc = slice(hh * 512, (hh + 1) * 512)
                            nc.tensor.matmul(sct[:, hc],
                                             lhsT=xk_sb[p][hr, lcol],
                                             rhs=xq_sb[p][hr, scol],
                                             start=True, stop=True)
                        if lt in DVE_LT:
                            pr_b = prb.tile([128, 1024], I16, tag="prb",
                                            name="prb")
                            nc.vector.tensor_scalar(pr_b, sct, EXP_A, EXP_B,
                                                    MUL, ADD)
                            prv = pr_b.bitcast(BF16)
                        else:
                            pr_a = pra.tile([128, 1024], BF16, tag="pra",
                                            name="pra")
                            nc.scalar.activation(pr_a, sct, Exp, scale=0.125)
                            prv = pr_a
                        # software pipeline: PV of the previous l-tile sits
                        # behind this tile's score matmuls in the PE queue
                        if prev is not None:
                            emit_pv(*prev)
                        prev = (prv, lt)
                    emit_pv(*prev)

                    # normalize straight from PV PSUM (ones-column gives the
                    # denominator in row 64).  All-DVE: gpsimd ucode ops
                    # (partition_broadcast) showed cold-start races on HW,
                    # so the broadcast is a stream_shuffle replicating rows
                    # 0/32 across each 32-partition quadrant.  recip input
                    # must be a partition-0 SBUF tile
                    # (reciprocal_approx_fast NaNs on HW otherwise)
                    for hh, pvt in ((0, pv0), (1, pv1)):
                        hrow = slice(64 * hh, 64 * hh + 64)
                        den = smal.tile([1, 512], F32, tag="den", name="den")
                        nc.vector.tensor_copy(den, pvt[HD:HD + 1, :])
                        rc = smal.tile([1, 512], F32, tag="rc", name="rc")
                        nc.vector.reciprocal_approx_fast(out=rc, in_=den)
                        rb = smal.tile([64, 512], F32, tag="rb", name="rb")
                        nc.gpsimd.partition_broadcast(rb, rc)
                        nc.vector.tensor_mul(attT[p][hrow, scol],
                                             pvt[0:HD, :], rb)

                    # WO folded one sb behind: attT(sb-1) is long complete,
                    # so these matmuls never stall on the normalize chain
                    if sb > 0:
                        emit_wo(4 * (sb - 1) + p)

            for p in range(NPAIR):
                emit_wo(12 + p)

        if DEBUG_DUMPS:
            dbg = aps[-1]
            nc.sync.dma_start(out=dbg["wq_all"], in_=wq_all)
            nc.sync.dma_start(out=dbg["xq0"], in_=xq_sb[0])
            nc.sync.dma_start(out=dbg["xk0"], in_=xk_sb[0])
            nc.sync.dma_start(out=dbg["xv0"], in_=xv_sb[0])
            nc.sync.dma_start(out=dbg["att0"], in_=attT[0])


def build_program():
    nc = bacc.Bacc("TRN2", target_bir_lowering=False, debug=False)
    qT = nc.dram_tensor("qT", [D, S], BF16, kind="ExternalInput").ap()
    kT = nc.dram_tensor("kT", [D, S], BF16, kind="ExternalInput").ap()
    vT = nc.dram_tensor("vT", [D, S], BF16, kind="ExternalInput").ap()
    wqT = nc.dram_tensor("wqT", [D, E], BF16, kind="ExternalInput").ap()
    wkT = nc.dram_tensor("wkT", [D, E], BF16, kind="ExternalInput").ap()
    wvT = nc.dram_tensor("wvT", [D, E], BF16, kind="ExternalInput").ap()
    woT = nc.dram_tensor("woT", [E, D], BF16, kind="ExternalInput").ap()
    ctab = nc.dram_tensor("ct", [128, S], BF16, kind="ExternalInput").ap()
    stab = nc.dram_tensor("st2", [128, S], BF16, kind="ExternalInput").ap()
    out = nc.dram_tensor("out", [S, D], F32, kind="ExternalOutput").ap()
    aps = (qT, kT, vT, wqT, wkT, wvT, woT, ctab, stab, out)
    if DEBUG_DUMPS:
        dbg = {
            "pv0": nc.dram_tensor("d_pv0", [HD, 512], F32,
                                  kind="ExternalOutput").ap(),
            "pv1": nc.dram_tensor("d_pv1", [HD, 512], F32,
                                  kind="ExternalOutput").ap(),
            "rc0": nc.dram_tensor("d_rc0", [1, 512], F32,
                                  kind="ExternalOutput").ap(),
            "rc1": nc.dram_tensor("d_rc1", [1, 512], F32,
                                  kind="ExternalOutput").ap(),
            "rb0": nc.dram_tensor("d_rb0", [64, 512], F32,
                                  kind="ExternalOutput").ap(),
            "rb1": nc.dram_tensor("d_rb1", [64, 512], F32,
                                  kind="ExternalOutput").ap(),
            "wq_all": nc.dram_tensor("d_wq", [128, ND * E], BF16,
                                     kind="ExternalOutput").ap(),
            "xq0": nc.dram_tensor("d_xq0", [128, S], BF16,
                                  kind="ExternalOutput").ap(),
            "xk0": nc.dram_tensor("d_xk0", [128, S], BF16,
                                  kind="ExternalOutput").ap(),
            "xv0": nc.dram_tensor("d_xv0", [128, HLOC * (HD + 1)], BF16,
                                  kind="ExternalOutput").ap(),
            "att0": nc.dram_tensor("d_att0", [128, S], BF16,
                                   kind="ExternalOutput").ap(),
        }
        aps = aps + (dbg,)
    with tile.TileContext(nc) as tc:
        _emit(nc, tc, aps)
    nc.compile()
    return nc


def host_prep(q, k, v, freqs_cis, wq, wk, wv, wo):
    """Build the 8 per-core input maps."""
    q = np.asarray(q, dtype=np.float32)
    k = np.asarray(k, dtype=np.float32)
    v = np.asarray(v, dtype=np.float32)
    fc = np.asarray(freqs_cis, dtype=np.float32)
    wq = np.asarray(wq, dtype=np.float32)
    wk = np.asarray(wk, dtype=np.float32)
    wv = np.asarray(wv, dtype=np.float32)
    wo = np.asarray(wo, dtype=np.float32)

    cos, sin = fc[:, :, 0], fc[:, :, 1]            # (S, 32)
    idx = (np.arange(128) % 64) // 2
    ct = np.ascontiguousarray(cos[:, idx].T)       # (128, S)
    sgn = np.where(np.arange(128) % 2 == 0, -1.0, 1.0).astype(np.float32)
    st = np.ascontiguousarray(sin[:, idx].T * sgn[:, None])
    st2 = -st                                      # row-swapped st

    def b16(a):
        return np.ascontiguousarray(a).astype(NPBF16)

    in_maps = []
    for c in range(NCORES):
        b, g = c // 2, c % 2
        rows = slice(g * E, (g + 1) * E)
        in_maps.append({
            "qT": b16(q[:, b, :].T),
            "kT": b16(k[:, b, :].T),
            "vT": b16(v[:, b, :].T),
            "wqT": b16(wq[rows, :].T),
            "wkT": b16(wk[rows, :].T),
            "wvT": b16(wv[rows, :].T),
            "woT": b16(wo[:, rows].T),
            "ct": b16(ct),
            "st2": b16(st2),
        })
    return in_maps


def kernel(q, k, v, freqs_cis, wq, wk, wv, wo, trace=False):
    global _PROG, LAST_RESULT
    if _PROG is None:
        _PROG = build_program()
    in_maps = host_prep(q, k, v, freqs_cis, wq, wk, wv, wo)
    res = run_bass_kernel_spmd(_PROG, in_maps, list(range(NCORES)), trace=trace)
    LAST_RESULT = res
    out = np.empty((S, B, D), dtype=np.float32)
    for b in range(B):
        out[:, b, :] = res.results[2 * b]["out"] + res.results[2 * b + 1]["out"]
    return out


# revision 61
# speedup vs baseline: 1.2229x; 1.2229x over previous
# Trainium2 Bass kernel for nn_Attention_63900523430102.
#
# Reference computes, for q,k,v of shape (S=2048, B=4, D=1024):
#   xq = q @ wq.T, xk = k @ wk.T, xv = v @ wv.T  (per-head split, hd=64, H=16)
#   xq, xk = rope(xq), rope(xk)
#   scores = softmax(xq . xk / sqrt(hd)) ; out = (scores @ xv) @ wo.T
#
# Sharding: 8 cores = 4 batches x 2 head-groups (8 heads each).  Host
# pretransposes operands so projections emit xqT/xkT [hd, S] directly.
#
# This version (vs the 525us baseline):
#  - scores matmuls are K=64 row-tiled: the two heads of a pair live in
#    partitions 0-63 / 64-127, and their score matmuls are emitted
#    back-to-back so the PE runs them CONCURRENTLY on the two halves of
#    the array (tile_position (0,0) / (64,0)) -> 2x score throughput.
#  - softmax exp is split across ACT (exact Exp) and DVE (Schraudolph
#    int16-bitcast exp approx, one tensor_scalar op) -- ACT alone is a
#    ~285us wall.
#  - phase order k-proj -> v-proj -> q-proj so attention starts as soon
#    as xq(sb0) exists; WO is folded per-sb reusing the score PSUM pool.
#  - batched DMAs (one trigger per [128, 4096] block) to cut trigger cost.
import sys
import os

sys.path.insert(0, "/opt/trn_rl_repo")

import numpy as np
import ml_dtypes

import concourse.bass as bass
import concourse.bacc as bacc
import concourse.mybir as mybir
import concourse.tile as tile
from concourse.bass_utils import run_bass_kernel_spmd


def _shim_axon_hooks():
    """Provide antenv.axon_hooks (NTFF profile hook) if the image lacks it."""
    try:
        from antenv.axon_hooks import get_axon_ntff_profile_hook  # noqa: F401
        return
    except ImportError:
        pass
    import types
    import ctypes
    import contextlib

    so_path = "/opt/axon/libaxon_pjrt.so"
    hook = None
    if os.path.exists(so_path):
        lib = ctypes.CDLL(so_path)
        if hasattr(lib, "axon_start_nrt_profile"):
            lib.axon_start_nrt_profile.argtypes = [
                ctypes.POINTER(ctypes.c_int64), ctypes.c_size_t]
            lib.axon_start_nrt_profile.restype = ctypes.c_int64
            lib.axon_stop_nrt_profile.argtypes = [ctypes.c_char_p]
            lib.axon_stop_nrt_profile.restype = ctypes.c_int64

            @contextlib.contextmanager
            def hook(output_dir, device_ids):
                import jax
                jax.devices()
                if device_ids:
                    ids = (ctypes.c_int64 * len(device_ids))(*device_ids)
                    rc = lib.axon_start_nrt_profile(ids, len(device_ids))
                else:
                    rc = lib.axon_start_nrt_profile(None, 0)
                if rc != 0:
                    raise RuntimeError(f"axon_start_nrt_profile rc={rc}")
                try:
                    yield
                finally:
                    n = lib.axon_stop_nrt_profile(str(output_dir).encode())
                    print(f"ntff profile: {n} file(s) -> {output_dir}",
                          file=sys.stderr)

    mod = types.ModuleType("antenv.axon_hooks")
    mod.get_axon_ntff_profile_hook = lambda: hook
    mod.set_axon_ntff_profile_hook = lambda h: None
    sys.modules["antenv.axon_hooks"] = mod


_shim_axon_hooks()

S = 2048          # seq len (both query s and key l)
D = 1024          # d_model
B = 4             # batch
HLOC = 8          # heads per core
HD = 64           # head dim
E = HLOC * HD     # 512, local e-width per core
NCORES = 8
NPAIR = 4         # head pairs per core (2 heads stacked -> 128 partitions)
NSB = 4           # s blocks of 512
NLT = 16          # l tiles of 128
ND = 8            # d_model tiles of 128

BF16 = mybir.dt.bfloat16
F32 = mybir.dt.float32
I16 = mybir.dt.int16
NPBF16 = ml_dtypes.bfloat16

# Schraudolph exp on DVE: pr = exp(0.125*sc) ~= bf16_bits(round(A*sc + B)).
# int16 value I interpreted as bf16 is 2^((I-16256)/128)*(1+frac), linear
# Schraudolph; B tuned for minimax relative error (~3.3% max, 2.0% rms).
EXP_A = 23.083120654223414
EXP_B = 16250.375
# which l-tile-pair groups keep the h1 exp on ACT (rebalance ACT vs DVE)
ACT_H1_G = (7,)

DEBUG_DUMPS = False

_PROG = None
LAST_RESULT = None


def _emit(nc, tc, aps):
    qT, kT, vT, wqT, wkT, wvT, woT, ctab, stab, out = aps[:10]
    Exp = mybir.ActivationFunctionType.Exp
    Copy = mybir.ActivationFunctionType.Copy
    MUL = mybir.AluOpType.mult
    ADD = mybir.AluOpType.add
    swap_mask = [i ^ 1 for i in range(32)]

    from contextlib import ExitStack

    with ExitStack() as stk:
        consts = stk.enter_context(tc.tile_pool(name="consts", bufs=1))
        persist = stk.enter_context(tc.tile_pool(name="persist", bufs=1))

        # ---- persistent constants ----
        # wk on sync, k tiles on gpsimd so the first projections' two
        # operands stream in parallel.
        wk_all = consts.tile([128, ND * E], BF16, tag="wk", name="wk")
        wv_all = consts.tile([128, ND * E], BF16, tag="wv", name="wv")
        wq_all = consts.tile([128, ND * E], BF16, tag="wq", name="wq")
        wo_all = consts.tile([128, NPAIR * D], BF16, tag="wo", name="wo")
        ct_t = consts.tile([128, S], BF16, tag="ct", name="ct")
        st2_t = consts.tile([128, S], BF16, tag="st2", name="st2")

        def load_w(dst, src, eng, nsplit=2):
            # dst [128, 8*512] <- src [1024, 512] (8 d-tiles), nsplit triggers
            d3 = dst.rearrange("p (d e) -> p d e", e=E)
            s3 = src.rearrange("(d p) e -> p d e", p=128)
            step = ND // nsplit
            for h in range(nsplit):
                eng.dma_start(out=d3[:, h * step:(h + 1) * step],
                              in_=s3[:, h * step:(h + 1) * step])

        load_w(wk_all, wkT, nc.sync)
        nc.sync.dma_start(out=ct_t, in_=ctab[:, :])
        nc.sync.dma_start(out=st2_t, in_=stab[:, :])
        load_w(wv_all, wvT, nc.sync)
        load_w(wq_all, wqT, nc.sync)

        # ---- persistent activations ----
        xq_sb = [persist.tile([128, S], BF16, tag=f"xq{p}", name=f"xq{p}")
                 for p in range(NPAIR)]
        xk_sb = [persist.tile([128, S], BF16, tag=f"xk{p}", name=f"xk{p}")
                 for p in range(NPAIR)]
        # xv with a ones column per head: head h occupies cols 65h..65h+64
        xv_sb = [persist.tile([128, HLOC * (HD + 1)], BF16,
                              tag=f"xv{lt}", name=f"xv{lt}")
                 for lt in range(NLT)]
        attT = [persist.tile([128, S], BF16, tag=f"att{p}", name=f"att{p}")
                for p in range(NPAIR)]

        # warm up the gpsimd partition_broadcast ucode: its very first
        # invocation on a cold core can signal completion before the write
        # lands (observed as intermittent first-call corruption).  Nothing
        # consumes wrm_b, so the real broadcasts all run with the ucode
        # resident.
        wrm_a = persist.tile([1, 512], F32, tag="wrm_a", name="wrm_a")
        wrm_b = persist.tile([64, 512], F32, tag="wrm_b", name="wrm_b")
        nc.vector.memset(wrm_a, 1.0)
        nc.gpsimd.partition_broadcast(wrm_b, wrm_a)

        # =============== Phase A: projections + rope ===============
        with ExitStack() as phase_a:
            kq_pool = phase_a.enter_context(tc.tile_pool(name="kq", bufs=4))
            v_pool = phase_a.enter_context(tc.tile_pool(name="vp", bufs=4))
            rp = phase_a.enter_context(tc.tile_pool(name="rope", bufs=2))
            psA = phase_a.enter_context(
                tc.tile_pool(name="psA", bufs=4, space="PSUM"))
            psV = phase_a.enter_context(
                tc.tile_pool(name="psV", bufs=3, space="PSUM"))

            def rope(ps, dst, sb):
                # dst[p] = ps[p]*ct[p] + ps[p^1]*st[p]; st2 = row-swapped st
                cols = slice(sb * 512, (sb + 1) * 512)
                xb = rp.tile([128, 512], BF16, tag="xb", name="xb")
                nc.scalar.activation(xb, ps, Copy)
                t1 = rp.tile([128, 512], BF16, tag="t1", name="t1")
                nc.vector.tensor_mul(t1, xb, ct_t[:, cols])
                t2 = rp.tile([128, 512], BF16, tag="t2", name="t2")
                nc.vector.tensor_mul(t2, xb, st2_t[:, cols])
                sw = rp.tile([128, 512], BF16, tag="sw", name="sw")
                nc.vector.stream_shuffle(sw, t2, swap_mask)
                nc.vector.tensor_add(dst[:, cols], t1, sw)

            def proj_qk(src_dram, w_all, dst_sb, dma_eng):
                for sb in range(NSB):
                    scol = slice(sb * 512, (sb + 1) * 512)
                    t = kq_pool.tile([128, ND * 512], BF16, tag="kq",
                                     name="kq")
                    d3 = t.rearrange("p (d e) -> p d e", e=512)
                    s3 = src_dram.rearrange("(d p) s -> p d s", p=128)
                    for h in range(2):
                        dma_eng.dma_start(
                            out=d3[:, h * 4:(h + 1) * 4],
                            in_=s3[:, h * 4:(h + 1) * 4, scol])
                    for p in range(NPAIR):
                        pc0 = p * 128
                        ps = psA.tile([128, 512], F32, tag="ps", name="ps")
                        for d in range(ND):
                            nc.tensor.matmul(
                                ps,
                                lhsT=w_all[:, d * E + pc0:d * E + pc0 + 128],
                                rhs=t[:, d * 512:(d + 1) * 512],
                                start=(d == 0), stop=(d == ND - 1))
                        rope(ps, dst_sb[p], sb)

            # k first (phase B sb0 needs all xk), then q, then v LAST:
            # the phase-A teardown barrier then waits only on the cheap v
            # drains instead of the 7us rope tail, and phase B's PV(g)
            # anyway consumes xv tiles in lt order.
            proj_qk(kT, wk_all, xk_sb, nc.gpsimd)
            proj_qk(qT, wq_all, xq_sb, nc.sync)

            for lt in range(NLT):
                lcol = slice(lt * 128, (lt + 1) * 128)
                vt = v_pool.tile([128, ND * 128], BF16, tag="v", name="v")
                nc.gpsimd.dma_start(
                    out=vt.rearrange("p (d l) -> p d l", l=128),
                    in_=vT.rearrange("(d p) s -> p d s", p=128)[:, :, lcol])
                xv_ps = psV.tile([128, 512], F32, tag="xv", name="xv")
                for d in range(ND):
                    nc.tensor.matmul(xv_ps,
                                     lhsT=vt[:, d * 128:(d + 1) * 128],
                                     rhs=wv_all[:, d * E:(d + 1) * E],
                                     start=(d == 0), stop=(d == ND - 1))
                dst = xv_sb[lt].rearrange("p (h c) -> p h c", c=HD + 1)
                src = xv_ps.rearrange("p (h c) -> p h c", c=HD)
                nc.vector.tensor_copy(dst[:, :, 0:HD], src)
                nc.vector.memset(dst[:, :, HD], 1.0)

        # wo load deferred: first consumed at the first WO round
        woc = wo_all.rearrange("p (e d) -> p e d", d=D)
        nc.sync.dma_start(out=woc, in_=woT.rearrange("(e p) d -> p e d", p=128))

        # ====== Phase B: attention (sb-major) with WO folded in ======
        # PSUM: scB 3x[128,1024] = 6 banks + pv0/pv1 = 2 banks.  WO reuses
        # the scB pool between sb rounds.
        with ExitStack() as phase_b:
            scB = phase_b.enter_context(
                tc.tile_pool(name="scB", bufs=3, space="PSUM"))
            pvP = phase_b.enter_context(
                tc.tile_pool(name="pvP", bufs=1, space="PSUM"))
            pra = phase_b.enter_context(tc.tile_pool(name="pra", bufs=4))
            prb = phase_b.enter_context(tc.tile_pool(name="prb", bufs=3))
            smal = phase_b.enter_context(tc.tile_pool(name="smal", bufs=2))
            otp = phase_b.enter_context(tc.tile_pool(name="otp", bufs=2))

            def emit_wo(sti):
                trow = slice(sti * 128, (sti + 1) * 128)
                ps = scB.tile([128, 1024], F32, tag="sc", name="wops")
                for nb in range(2):
                    nbc = slice(nb * 512, (nb + 1) * 512)
                    for et in range(NPAIR):
                        nc.tensor.matmul(
                            ps[:, nbc],
                            lhsT=attT[et][:, trow],
                            rhs=wo_all[:, et * D + nb * 512:
                                       et * D + nb * 512 + 512],
                            start=(et == 0), stop=(et == NPAIR - 1))
                ot = otp.tile([128, 1024], F32, tag="ot", name="ot")
                if sti % 2 == 0:
                    nc.scalar.activation(ot, ps, Copy)
                else:
                    nc.vector.tensor_copy(ot, ps)
                eng = nc.sync if sti % 2 == 0 else nc.gpsimd
                eng.dma_start(out=out[trow, :], in_=ot)

            for sb in range(NSB):
                scol = slice(sb * 512, (sb + 1) * 512)
                for p in range(NPAIR):
                    h0, h1 = 2 * p, 2 * p + 1
                    pv0 = pvP.tile([128, 512], F32, tag="pv0", name="pv0")
                    pv1 = pvP.tile([128, 512], F32, tag="pv1", name="pv1")

                    def emit_pv(pr_a, pr_bv, lt0):
                        for j in range(2):
                            lt = lt0 + j
                            jc = slice(j * 512, (j + 1) * 512)
                            nc.tensor.matmul(
                                pv0[0:HD + 1, :],
                                lhsT=xv_sb[lt][:, 65 * h0:65 * h0 + 65],
                                rhs=pr_a[:, jc],
                                start=(lt == 0), stop=(lt == NLT - 1))
                            nc.tensor.matmul(
                                pv1[0:HD + 1, :],
                                lhsT=xv_sb[lt][:, 65 * h1:65 * h1 + 65],
                                rhs=pr_bv[:, jc],
                                start=(lt == 0), stop=(lt == NLT - 1))

                    prev = None
                    for g in range(NLT // 2):
                        lt0 = 2 * g
                        sc_a = scB.tile([128, 1024], F32, tag="sc", name="sa")
                        sc_b = scB.tile([128, 1024], F32, tag="sc", name="sb")
                        # interleave the two heads' matmuls: row-tiles
                        # (0,0)/(64,0) run concurrently on the PE array.
                        # ABBA order: MM starts are strict FIFO, so abab
                        # would serialize the second pair behind a-j1.
                        for j, hh in ((0, 0), (0, 1), (1, 1), (1, 0)):
                            lcol = slice((lt0 + j) * 128, (lt0 + j + 1) * 128)
                            jc = slice(j * 512, (j + 1) * 512)
                            hr = slice(64 * hh, 64 * hh + 64)
                            nc.tensor.matmul(sc_a[:, jc] if hh == 0
                                             else sc_b[:, jc],
                                             lhsT=xk_sb[p][hr, lcol],
                                             rhs=xq_sb[p][hr, scol],
                                             start=True, stop=True)
                        pr_a = pra.tile([128, 1024], BF16, tag="pra",
                                        name="pra")
                        nc.scalar.activation(pr_a, sc_a, Exp, scale=0.125)
                        if g in ACT_H1_G:
                            pr_bv = pra.tile([128, 1024], BF16, tag="pra",
                                             name="prb_act")
                            nc.scalar.activation(pr_bv, sc_b, Exp,
                                                 scale=0.125)
                        else:
                            pr_b = prb.tile([128, 1024], I16, tag="prb",
                                            name="prb")
                            nc.vector.tensor_scalar(pr_b, sc_b, EXP_A, EXP_B,
                                                    MUL, ADD)
                            pr_bv = pr_b.bitcast(BF16)
                        # software pipeline: PV of the previous group sits
                        # behind this group's score matmuls in the PE queue
                        if prev is not None:
                            emit_pv(*prev)
                        prev = (pr_a, pr_bv, lt0)
                    emit_pv(*prev)

                    # normalize straight from PV PSUM (ones-column gives the
                    # denominator in row 64).  All-DVE: gpsimd ucode ops
                    # (partition_broadcast) showed cold-start races on HW,
                    # so the broadcast is a stream_shuffle replicating rows
                    # 0/32 across each 32-partition quadrant.  recip input
                    # must be a partition-0 SBUF tile
                    # (reciprocal_approx_fast NaNs on HW otherwise)
                    for hh, pvt in ((0, pv0), (1, pv1)):
                        hrow = slice(64 * hh, 64 * hh + 64)
                        den = smal.tile([1, 512], F32, tag="den", name="den")
                        nc.vector.tensor_copy(den, pvt[HD:HD + 1, :])
                        rc = smal.tile([1, 512], F32, tag="rc", name="rc")
                        nc.vector.reciprocal_approx_fast(out=rc, in_=den)
                        rb = smal.tile([64, 512], F32, tag="rb", name="rb")
                        nc.gpsimd.partition_broadcast(rb, rc)
                        nc.vector.tensor_mul(attT[p][hrow, scol],
                                             pvt[0:HD, :], rb)

                    # WO folded one sb behind: attT(sb-1) is long complete,
                    # so these matmuls never stall on the normalize chain
                    if sb > 0:
                        emit_wo(4 * (sb - 1) + p)

            for p in range(NPAIR):
                emit_wo(12 + p)

        if DEBUG_DUMPS:
            dbg = aps[-1]
            nc.sync.dma_start(out=dbg["wq_all"], in_=wq_all)
            nc.sync.dma_start(out=dbg["xq0"], in_=xq_sb[0])
            nc.sync.dma_start(out=dbg["xk0"], in_=xk_sb[0])
            nc.sync.dma_start(out=dbg["xv0"], in_=xv_sb[0])
            nc.sync.dma_start(out=dbg["att0"], in_=attT[0])


def build_program():
    nc = bacc.Bacc("TRN2", target_bir_lowering=False, debug=False)
    qT = nc.dram_tensor("qT", [D, S], BF16, kind="ExternalInput").ap()
    kT = nc.dram_tensor("kT", [D, S], BF16, kind="ExternalInput").ap()
    vT = nc.dram_tensor("vT", [D, S], BF16, kind="ExternalInput").ap()
    wqT = nc.dram_tensor("wqT", [D, E], BF16, kind="ExternalInput").ap()
    wkT = nc.dram_tensor("wkT", [D, E], BF16, kind="ExternalInput").ap()
    wvT = nc.dram_tensor("wvT", [D, E], BF16, kind="ExternalInput").ap()
    woT = nc.dram_tensor("woT", [E, D], BF16, kind="ExternalInput").ap()
    ctab = nc.dram_tensor("ct", [128, S], BF16, kind="ExternalInput").ap()
    stab = nc.dram_tensor("st2", [128, S], BF16, kind="ExternalInput").ap()
    out = nc.dram_tensor("out", [S, D], F32, kind="ExternalOutput").ap()
    aps = (qT, kT, vT, wqT, wkT, wvT, woT, ctab, stab, out)
    if DEBUG_DUMPS:
        dbg = {
            "pv0": nc.dram_tensor("d_pv0", [HD, 512], F32,
                                  kind="ExternalOutput").ap(),
            "pv1": nc.dram_tensor("d_pv1", [HD, 512], F32,
                                  kind="ExternalOutput").ap(),
            "rc0": nc.dram_tensor("d_rc0", [1, 512], F32,
                                  kind="ExternalOutput").ap(),
            "rc1": nc.dram_tensor("d_rc1", [1, 512], F32,
                                  kind="ExternalOutput").ap(),
            "rb0": nc.dram_tensor("d_rb0", [64, 512], F32,
                                  kind="ExternalOutput").ap(),
            "rb1": nc.dram_tensor("d_rb1", [64, 512], F32,
                                  kind="ExternalOutput").ap(),
            "wq_all": nc.dram_tensor("d_wq", [128, ND * E], BF16,
                                     kind="ExternalOutput").ap(),
            "xq0": nc.dram_tensor("d_xq0", [128, S], BF16,
                                  kind="ExternalOutput").ap(),
            "xk0": nc.dram_tensor("d_xk0", [128, S], BF16,
                                  kind="ExternalOutput").ap(),
            "xv0": nc.dram_tensor("d_xv0", [128, HLOC * (HD + 1)], BF16,
                                  kind="ExternalOutput").ap(),
            "att0": nc.dram_tensor("d_att0", [128, S], BF16,
                                   kind="ExternalOutput").ap(),
        }
        aps = aps + (dbg,)
    with tile.TileContext(nc) as tc:
        _emit(nc, tc, aps)
    nc.compile()
    return nc


def host_prep(q, k, v, freqs_cis, wq, wk, wv, wo):
    """Build the 8 per-core input maps."""
    q = np.asarray(q, dtype=np.float32)
    k = np.asarray(k, dtype=np.float32)
    v = np.asarray(v, dtype=np.float32)
    fc = np.asarray(freqs_cis, dtype=np.float32)
    wq = np.asarray(wq, dtype=np.float32)
    wk = np.asarray(wk, dtype=np.float32)
    wv = np.asarray(wv, dtype=np.float32)
    wo = np.asarray(wo, dtype=np.float32)

    cos, sin = fc[:, :, 0], fc[:, :, 1]            # (S, 32)
    idx = (np.arange(128) % 64) // 2
    ct = np.ascontiguousarray(cos[:, idx].T)       # (128, S)
    sgn = np.where(np.arange(128) % 2 == 0, -1.0, 1.0).astype(np.float32)
    st = np.ascontiguousarray(sin[:, idx].T * sgn[:, None])
    st2 = -st                                      # row-swapped st

    def b16(a):
        return np.ascontiguousarray(a).astype(NPBF16)

    in_maps = []
    for c in range(NCORES):
        b, g = c // 2, c % 2
        rows = slice(g * E, (g + 1) * E)
        in_maps.append({
            "qT": b16(q[:, b, :].T),
            "kT": b16(k[:, b, :].T),
            "vT": b16(v[:, b, :].T),
            "wqT": b16(wq[rows, :].T),
            "wkT": b16(wk[rows, :].T),
            "wvT": b16(wv[rows, :].T),
            "woT": b16(wo[:, rows].T),
            "ct": b16(ct),
            "st2": b16(st2),
        })
    return in_maps


def kernel(q, k, v, freqs_cis, wq, wk, wv, wo, trace=False):
    global _PROG, LAST_RESULT
    if _PROG is None:
        _PROG = build_program()
    in_maps = host_prep(q, k, v, freqs_cis, wq, wk, wv, wo)
    res = run_bass_kernel_spmd(_PROG, in_maps, list(range(NCORES)), trace=trace)
    LAST_RESULT = res
    out = np.empty((S, B, D), dtype=np.float32)
    for b in range(B):
        out[:, b, :] = res.results[2 * b]["out"] + res.results[2 * b + 1]["out"]
    return out
